# revision 1
# baseline (speedup 1.0000x reference)
"""Swin-style windowed local self-attention (LN -> QKV -> 7x7 window MHA
with relative position bias -> proj) on 8 Trainium2 NeuronCores.

Sharding: pure data parallel over B*T (24 images -> 3 per core).

v2 design -- minimize instruction count and DMA count; everything
feature-major until the last possible moment:
  - host ships x already window-ordered, TRANSPOSED (feature-major) and
    cast to bf16: xT [384, 9408] per core.  LN gamma/scale are folded
    into the QKV weights on host.
  - LN is computed ON DEVICE from xT: per-token sums/sumsq via
    ones-vector matmuls (PE), row math on [1,392] tiles (DVE), invstd =
    sqrt(1/(var+eps)) (DVE reciprocal + ACT sqrt), then xT is normalized
    IN PLACE feature-major with partition-broadcast row vectors.
  - q/k are projected feature-major in 392-token chunks (N=392 matmuls).
  - windows are processed in PAIRS (98 tokens).  Scores for both windows
    of a pair are computed in ONE matmul per head ([32,98] x [32,98] ->
    [98,98]); the cross-window blocks are garbage that is masked by the
    relative-position-bias matmul (bias table carries -30000 in the
    cross blocks, so exp -> exactly 0).
  - the bias (+mask) is accumulated into the score PSUM FIRST
    (start=True) via an identity-weight matmul (one per 4-head quad),
    then the 4 head matmuls accumulate on top.
  - softmax: exp on ACT; denominators Z via a ones-column matmul per
    quad ([1,392] rows); reciprocal on DVE; P normalized in place with
    one [98,1176] broadcast multiply.
  - P@V is computed TRANSPOSED: lhsT = v (token-major), rhs = P ->
    A^T [32,98] per head, packed 4 heads per PSUM tile via column
    tiling.  The evicted [128, 98] tiles are directly the lhsT of the
    projection matmul -- no transposes anywhere in the kernel.
  - v is computed token-major per pair (3 matmuls, both windows at
    once), proj likewise (3 matmuls), output stored with one DMA/pair.
"""

import sys

if "/opt/trn_rl_repo" not in sys.path:
    sys.path.insert(0, "/opt/trn_rl_repo")

import numpy as np
import ml_dtypes

import concourse.bacc as bacc
import concourse.bass as bass
import concourse.bass_isa as bass_isa
import concourse.tile as tile
import concourse.mybir as mybir
from concourse.bass_utils import run_bass_kernel_spmd

F32 = mybir.dt.float32
BF16 = mybir.dt.bfloat16

N_CORES = 8
B, T, H, W, D = 4, 6, 56, 56, 384
WSZ = 7
NH = 12
HD = D // NH            # 32
N = WSZ * WSZ           # 49 tokens / window
IMGS_CORE = (B * T) // N_CORES     # 3
TOK_CORE = IMGS_CORE * H * W       # 9408
NPAIR = TOK_CORE // (2 * N)        # 96 window pairs / core
PT = 2 * N                         # 98 tokens / pair
EPS = 1e-5

N_GROUPS = 4
PAIRS_G = NPAIR // N_GROUPS        # 24
TOK_G = PAIRS_G * PT               # 2352
CH = 4 * PT                        # 392-token chunks (stats + q/k)
NCH_G = TOK_G // CH                # 6 chunks / group
NCH = NPAIR * PT // CH             # 24 chunks / core
MASKVAL = -30000.0


def _rel_index(w):
    coords = np.stack(np.meshgrid(np.arange(w), np.arange(w), indexing="ij")).reshape(2, -1)
    rel = coords[:, :, None] - coords[:, None, :]
    return (rel[0] + w - 1) * (2 * w - 1) + (rel[1] + w - 1)


def _bcast(sl, parts):
    """partition-broadcast of a [1, n] row slice to [parts, n]"""
    return bass.AP(tensor=sl.tensor, offset=sl.offset,
                   ap=[[0, parts], *sl.ap[1:]])


def build_program(repeat=1, with_bias=False, n_groups=N_GROUPS, n_stats=NCH, pairs_g=PAIRS_G,
                  stage_upto=5, z_mode="gpsimd"):
    nc = bacc.Bacc("TRN2", target_bir_lowering=False, debug=False, num_devices=N_CORES)

    xT_d = nc.dram_tensor("xT", [D, TOK_CORE], BF16, kind="ExternalInput")
    qkw_d = nc.dram_tensor("qkw", [D, 2 * D], BF16, kind="ExternalInput")
    vw_d = nc.dram_tensor("vw", [D, D], BF16, kind="ExternalInput")
    pw_d = nc.dram_tensor("pw", [D, D], BF16, kind="ExternalInput")
    cqkr_d = nc.dram_tensor("cqkr", [2 * D], BF16, kind="ExternalInput")
    cvr_d = nc.dram_tensor("cvr", [D], BF16, kind="ExternalInput")
    pbr_d = nc.dram_tensor("pbr", [D], BF16, kind="ExternalInput")
    b2_d = nc.dram_tensor("b2", [4, PT, 3 * PT], BF16, kind="ExternalInput")
    i98_d = nc.dram_tensor("i98", [PT, PT], BF16, kind="ExternalInput")
    y_d = nc.dram_tensor("y", [TOK_CORE, D], F32, kind="ExternalOutput")

    from contextlib import ExitStack
    with tile.TileContext(nc) as tc, ExitStack() as ctx:
        const = ctx.enter_context(tc.tile_pool(name="const", bufs=1))
        grp = ctx.enter_context(tc.tile_pool(name="grp", bufs=2))
        work = ctx.enter_context(tc.tile_pool(name="work", bufs=4))
        row = ctx.enter_context(tc.tile_pool(name="row", bufs=2))
        ps_gen = ctx.enter_context(tc.tile_pool(name="ps_gen", bufs=2, space="PSUM"))
        ps_sc = ctx.enter_context(tc.tile_pool(name="ps_sc", bufs=4, space="PSUM"))
        ps_misc = ctx.enter_context(tc.tile_pool(name="ps_misc", bufs=2, space="PSUM"))

        # ---- resident constants -------------------------------------------------
        xT = [const.tile([128, TOK_CORE], BF16, name=f"xT{k}", tag=f"xT{k}") for k in range(3)]
        for k in range(3):
            nc.sync.dma_start(out=xT[k][:], in_=xT_d[128 * k:128 * (k + 1), :])
        qkw_sb = [const.tile([128, 2 * D], BF16, name=f"qkw{k}", tag=f"qkw{k}") for k in range(3)]
        vw_sb = [const.tile([128, D], BF16, name=f"vw{k}", tag=f"vw{k}") for k in range(3)]
        pw_sb = [const.tile([128, D], BF16, name=f"pw{k}", tag=f"pw{k}") for k in range(3)]
        for k in range(3):
            nc.sync.dma_start(out=qkw_sb[k][:], in_=qkw_d[128 * k:128 * (k + 1), :])
            nc.sync.dma_start(out=vw_sb[k][:], in_=vw_d[128 * k:128 * (k + 1), :])
            nc.sync.dma_start(out=pw_sb[k][:], in_=pw_d[128 * k:128 * (k + 1), :])
        cqkr_sb = const.tile([1, 2 * D], BF16, name="cqkr", tag="cqkr")
        nc.sync.dma_start(out=cqkr_sb[:], in_=cqkr_d[:])
        cvr_sb = const.tile([1, D], BF16, name="cvr", tag="cvr")
        nc.sync.dma_start(out=cvr_sb[:], in_=cvr_d[:])
        pbr_sb = const.tile([1, D], BF16, name="pbr", tag="pbr")
        nc.sync.dma_start(out=pbr_sb[:], in_=pbr_d[:])
        b2_sb = [const.tile([PT, 3 * PT], BF16, name=f"b2_{j}", tag=f"b2_{j}") for j in range(4)]
        for j in range(4):
            nc.sync.dma_start(out=b2_sb[j][:], in_=b2_d[j])
        i98_sb = const.tile([PT, PT], BF16, name="i98", tag="i98")
        nc.sync.dma_start(out=i98_sb[:], in_=i98_d[:])
        ones_sb = const.tile([128, 1], BF16, name="ones", tag="ones")
        nc.vector.memset(ones_sb[:], 1.0)
        onesr_sb = const.tile([1, 512], BF16, name="onesr", tag="onesr")
        nc.vector.memset(onesr_sb[:], 1.0)
        onessq_sb = const.tile([PT, PT], BF16, name="onessq", tag="onessq")
        nc.vector.memset(onessq_sb[:], 1.0)

        rep_ctx = tc.For_i(0, repeat, 1) if repeat > 1 else None
        if rep_ctx is not None:
            rep_ctx.__enter__()

        # ---- stage 1: LN stats + in-place feature-major normalize ---------------
        for c in range(n_stats):
            c0 = CH * c
            zs = ps_misc.tile([1, CH], F32, name="zs", tag="zrow")
            zq = ps_misc.tile([1, CH], F32, name="zq", tag="zrow")
            for k in range(3):
                sq = work.tile([128, CH], BF16, name="xsq", tag="xsq")
                nc.vector.tensor_tensor(
                    out=sq[:], in0=xT[k][:, c0:c0 + CH], in1=xT[k][:, c0:c0 + CH],
                    op=mybir.AluOpType.mult,
                )
                nc.tensor.matmul(zs[:], lhsT=ones_sb[:], rhs=xT[k][:, c0:c0 + CH],
                                 start=(k == 0), stop=(k == 2), tile_position=(0, 0))
                nc.tensor.matmul(zq[:], lhsT=ones_sb[:], rhs=sq[:],
                                 start=(k == 0), stop=(k == 2), tile_position=(0, 0))
            mu = row.tile([1, CH], BF16, name="mu", tag="mu")
            nc.vector.tensor_scalar(out=mu[:], in0=zs[:], scalar1=1.0 / D, scalar2=None,
                                    op0=mybir.AluOpType.mult)
            m2 = row.tile([1, CH], F32, name="m2", tag="m2")
            nc.vector.tensor_tensor(out=m2[:], in0=mu[:], in1=mu[:],
                                    op=mybir.AluOpType.mult)
            var = row.tile([1, CH], F32, name="var", tag="var")
            # var+eps = zq/D + eps - mu^2
            nc.vector.tensor_scalar(out=var[:], in0=zq[:], scalar1=1.0 / D, scalar2=EPS,
                                    op0=mybir.AluOpType.mult, op1=mybir.AluOpType.add)
            nc.vector.tensor_tensor(out=var[:], in0=var[:], in1=m2[:],
                                    op=mybir.AluOpType.subtract)
            nc.vector.reciprocal(out=var[:], in_=var[:])
            isr = row.tile([1, CH], BF16, name="isr", tag="isr")
            nc.scalar.activation(out=isr[:], in_=var[:],
                                 func=mybir.ActivationFunctionType.Sqrt)
            # broadcast the rows across partitions on the (idle) GPSIMD engine
            mu_bc = work.tile([128, CH], BF16, name="mu_bc", tag="mu_bc")
            nc.gpsimd.partition_broadcast(mu_bc[:], mu[:])
            isr_bc = work.tile([128, CH], BF16, name="isr_bc", tag="isr_bc")
            nc.gpsimd.partition_broadcast(isr_bc[:], isr[:])
            for k in range(3):
                nc.vector.tensor_tensor(
                    out=xT[k][:, c0:c0 + CH], in0=xT[k][:, c0:c0 + CH],
                    in1=mu_bc[:], op=mybir.AluOpType.subtract,
                )
                nc.vector.tensor_tensor(
                    out=xT[k][:, c0:c0 + CH], in0=xT[k][:, c0:c0 + CH],
                    in1=isr_bc[:], op=mybir.AluOpType.mult,
                )

        for g in range(n_groups):
            T0 = TOK_G * g
            qk = [grp.tile([128, TOK_G], BF16, name=f"qk{m}", tag=f"qk{m}") for m in range(6)]

            # ---- phase B: q/k projections (feature-major) ----------------------
            for cc in range(NCH_G):
                c0 = T0 + CH * cc
                for m in range(6):
                    pq = ps_gen.tile([128, CH], F32, name="pq", tag="gen")
                    if with_bias:
                        nc.tensor.matmul(
                            pq[:], lhsT=cqkr_sb[0:1, 128 * m:128 * (m + 1)],
                            rhs=onesr_sb[0:1, 0:CH],
                            start=True, stop=False, tile_position=(0, 0),
                        )
                    for k in range(3):
                        nc.tensor.matmul(
                            pq[:],
                            lhsT=qkw_sb[k][:, 128 * m:128 * (m + 1)],
                            rhs=xT[k][:, c0:c0 + CH],
                            start=(k == 0 and not with_bias), stop=(k == 2),
                            tile_position=(0, 0),
                        )
                    nc.any.tensor_copy(out=qk[m][:, CH * cc:CH * (cc + 1)], in_=pq[:])

            # ---- phase C: per window pair --------------------------------------
            for p in range(pairs_g):
                t0 = T0 + PT * p      # global token col (xT)
                qc = PT * p           # within-group token col (qk)

                # v for both windows: [98, 384] token-major (+cv via rank-1)
                pv = ps_gen.tile([PT, D], F32, name="pv", tag="gen")
                if with_bias:
                    nc.tensor.matmul(pv[:], lhsT=onesr_sb[0:1, 0:PT], rhs=cvr_sb[:],
                                     start=True, stop=False, tile_position=(0, 0))
                for k in range(3):
                    nc.tensor.matmul(
                        pv[:], lhsT=xT[k][:, t0:t0 + PT], rhs=vw_sb[k][:],
                        start=(k == 0 and not with_bias), stop=(k == 2),
                        tile_position=(0, 0),
                    )
                av = work.tile([PT, D], BF16, name="av", tag="av")
                nc.any.tensor_copy(out=av[:], in_=pv[:])
                if stage_upto < 2:
                    continue

                # scores, STRIP-major: PSUM bank j holds heads {j, 4+j, 8+j}
                # so each bank is only ever written by row-strip j -- two
                # different row tiles must never write one PSUM bank
                # concurrently (HW restriction).  Bias (+cross-window mask)
                # goes in first via a full-array identity matmul.
                p_t = work.tile([PT, NH * PT], BF16, name="pt", tag="pt")
                sc_j = []
                for j in range(4):
                    sc = ps_sc.tile([PT, 3 * PT], F32, name="sc", tag="sc")
                    nc.tensor.matmul(sc[:], lhsT=i98_sb[:], rhs=b2_sb[j][:],
                                     start=True, stop=False, tile_position=(0, 0))
                    sc_j.append(sc)
                for j in range(4):
                    hb = 32 * j
                    for quad in range(3):
                        nc.tensor.matmul(
                            sc_j[j][:, PT * quad:PT * (quad + 1)],
                            lhsT=qk[3 + quad][hb:hb + 32, qc:qc + PT],
                            rhs=qk[quad][hb:hb + 32, qc:qc + PT],
                            start=False, stop=(quad == 2), tile_position=(hb, 0),
                        )
                for j in range(4):
                    nc.scalar.activation(
                        out=p_t[:, 3 * PT * j:3 * PT * (j + 1)], in_=sc_j[j][:],
                        func=mybir.ActivationFunctionType.Exp,
                    )

                if stage_upto < 3:
                    continue
                # softmax denominators: partition all-reduce on GPSIMD gives Z
                # already broadcast; reciprocal on DVE; bf16 2x-mode multiply
                # normalizes p_t in place.
                for j in range(4):
                    if z_mode == "gpsimd":
                        zbc = work.tile([PT, 3 * PT], BF16, name="zbc", tag="zbc")
                        nc.gpsimd.partition_all_reduce(
                            zbc[:], p_t[:, 3 * PT * j:3 * PT * (j + 1)],
                            channels=PT, reduce_op=bass_isa.ReduceOp.add,
                        )
                    else:
                        zps = ps_sc.tile([PT, 3 * PT], F32, name="zps", tag="sc")
                        nc.tensor.matmul(zps[:], lhsT=onessq_sb[:],
                                         rhs=p_t[:, 3 * PT * j:3 * PT * (j + 1)],
                                         start=True, stop=True, tile_position=(0, 0))
                        zbc = zps
                    rb = work.tile([PT, 3 * PT], BF16, name="rb", tag="rb")
                    with nc.allow_low_precision(reason="1/Z bf16; feeds bf16 2x multiply"):
                        nc.vector.reciprocal(out=rb[:], in_=zbc[:])
                    nc.vector.tensor_tensor(
                        out=p_t[:, 3 * PT * j:3 * PT * (j + 1)],
                        in0=p_t[:, 3 * PT * j:3 * PT * (j + 1)],
                        in1=rb[:], op=mybir.AluOpType.mult,
                    )

                if stage_upto < 4:
                    continue
                # P@V transposed: A^T [32,98] per head, 4 heads / PSUM tile
                # (full-contraction matmuls, column-tiled -- no row strips)
                atT = work.tile([128, 3 * PT], BF16, name="atT", tag="atT")
                for quad in range(3):
                    ao = ps_misc.tile([128, PT], F32, name="ao", tag="zrow")
                    for j in range(4):
                        h = 4 * quad + j
                        pcol = 3 * PT * (h % 4) + PT * (h // 4)
                        nc.tensor.matmul(
                            ao[32 * j:32 * (j + 1), :],
                            lhsT=av[:, HD * h:HD * (h + 1)],
                            rhs=p_t[:, pcol:pcol + PT],
                            start=True, stop=True, tile_position=(0, 32 * j),
                        )
                    nc.any.tensor_copy(out=atT[:, PT * quad:PT * (quad + 1)], in_=ao[:])

                if stage_upto < 5:
                    continue
                # proj (+pb via rank-1) + store
                pp = ps_gen.tile([PT, D], F32, name="pp", tag="gen")
                if with_bias:
                    nc.tensor.matmul(pp[:], lhsT=onesr_sb[0:1, 0:PT], rhs=pbr_sb[:],
                                     start=True, stop=False, tile_position=(0, 0))
                for k in range(3):
                    nc.tensor.matmul(
                        pp[:], lhsT=atT[:, PT * k:PT * (k + 1)], rhs=pw_sb[k][:],
                        start=(k == 0 and not with_bias), stop=(k == 2),
                        tile_position=(0, 0),
                    )
                y_sb = work.tile([PT, D], F32, name="y", tag="y")
                nc.any.tensor_copy(out=y_sb[:], in_=pp[:])
                r0 = T0 + PT * p
                nc.sync.dma_start(out=y_d[r0:r0 + PT, :], in_=y_sb[:])

        if rep_ctx is not None:
            rep_ctx.__exit__(None, None, None)

    nc.compile()
    return nc


_NC_CACHE = {}


def _get_program(with_bias=False):
    key = ("nc", with_bias)
    if key not in _NC_CACHE:
        _NC_CACHE[key] = build_program(with_bias=with_bias)
    return _NC_CACHE[key]


def _window_order(xf):
    # [BT, H, W, D] -> [BT*nW*N, D] in window-raster order
    BT = xf.shape[0]
    x6 = xf.reshape(BT, H // WSZ, WSZ, W // WSZ, WSZ, D)
    return np.ascontiguousarray(x6.transpose(0, 1, 3, 2, 4, 5)).reshape(-1, D)


def _window_unorder(yw):
    BT = B * T
    y6 = yw.reshape(BT, H // WSZ, W // WSZ, WSZ, WSZ, D)
    return np.ascontiguousarray(y6.transpose(0, 1, 3, 2, 4, 5)).reshape(BT, H, W, D)


def prepare_inputs(x, ln_g, ln_b, qkv_w, qkv_b, proj_w, proj_b, rel_bias_table):
    x = np.asarray(x, np.float32)
    ln_g = np.asarray(ln_g, np.float32)
    ln_b = np.asarray(ln_b, np.float32)
    qkv_w = np.asarray(qkv_w, np.float32)
    qkv_b = np.asarray(qkv_b, np.float32)
    proj_w = np.asarray(proj_w, np.float32)
    proj_b = np.asarray(proj_b, np.float32)
    rel_bias_table = np.asarray(rel_bias_table, np.float32)

    scale = HD ** -0.5
    wq = qkv_w[:, :D] * ln_g[:, None] * scale
    wk = qkv_w[:, D:2 * D] * ln_g[:, None]
    wv = qkv_w[:, 2 * D:] * ln_g[:, None]
    cq = (ln_b @ qkv_w[:, :D] + qkv_b[:D]) * scale
    ck = ln_b @ qkv_w[:, D:2 * D] + qkv_b[D:2 * D]
    cv = ln_b @ qkv_w[:, 2 * D:] + qkv_b[2 * D:]

    qkw = np.concatenate([wq, wk], axis=1).astype(ml_dtypes.bfloat16)
    cqk = np.concatenate([cq, ck]).astype(np.float32)

    idx = _rel_index(WSZ)
    bias = rel_bias_table[idx.reshape(-1)].reshape(N, N, NH)  # [q, k, h]
    # strip-major bias tables: table j holds heads {j, 4+j, 8+j}
    b2 = np.full((4, PT, 3 * PT), MASKVAL, np.float32)
    for j in range(4):
        for quad in range(3):
            h = 4 * quad + j
            blkT = bias[:, :, h].T  # [k, q]
            for w in range(2):
                b2[j, N * w:N * (w + 1), PT * quad + N * w:PT * quad + N * (w + 1)] = blkT

    xw = _window_order(x.reshape(B * T, H, W, D))

    common = {
        "qkw": qkw,
        "vw": wv.astype(ml_dtypes.bfloat16),
        "pw": proj_w.astype(ml_dtypes.bfloat16),
        "cqkr": cqk.astype(ml_dtypes.bfloat16),
        "cvr": cv.astype(ml_dtypes.bfloat16),
        "pbr": proj_b.astype(ml_dtypes.bfloat16),
        "b2": b2.astype(ml_dtypes.bfloat16),
        "i98": np.eye(PT, dtype=np.float32).astype(ml_dtypes.bfloat16),
    }
    in_maps = []
    for c in range(N_CORES):
        m = dict(common)
        xc = xw[TOK_CORE * c:TOK_CORE * (c + 1)]
        m["xT"] = np.ascontiguousarray(xc.T).astype(ml_dtypes.bfloat16)
        in_maps.append(m)
    return in_maps


def kernel(x, ln_g, ln_b, qkv_w, qkv_b, proj_w, proj_b, rel_bias_table):
    in_maps = prepare_inputs(x, ln_g, ln_b, qkv_w, qkv_b, proj_w, proj_b, rel_bias_table)
    # the three affine constants (qkv bias, v bias, proj bias incl. folded
    # ln_b) are applied via rank-1 matmuls only when nonzero
    with_bias = any(
        np.any(np.asarray(in_maps[0][k], np.float32) != 0.0)
        for k in ("cqkr", "cvr", "pbr")
    )
    nc = _get_program(with_bias=with_bias)
    res = run_bass_kernel_spmd(nc, in_maps, core_ids=list(range(N_CORES)))
    yw = np.concatenate([res.results[c]["y"] for c in range(N_CORES)], axis=0)
    out = _window_unorder(yw).reshape(B, T, H, W, D)
    return out.astype(np.float32)



# revision 2
# speedup vs baseline: 7504.1607x; 7504.1607x over previous
"""Swin-style windowed local self-attention (LN -> QKV -> 7x7 window MHA
with relative position bias -> proj) on 8 Trainium2 NeuronCores.

Sharding: pure data parallel over B*T (24 images -> 3 per core).

v3 design -- engine-balance rework of v2.  Differences:
  - LN stats are computed in BROADCAST form: all-ones [128,128] lhsT
    matmuls give per-token sum / sumsq replicated across all 128
    partitions directly in PSUM, so the per-token mean/var math runs as
    [128, 392] multi-lane ops (no single-lane [1,N] row ops, no gpsimd
    partition broadcasts).
  - scores use 3 PSUM banks of [98, 392] (bank j holds heads {j, 3+j,
    6+j, 9+j}); bias+mask is accumulated first via an identity-weight
    matmul, 3 exp ops per pair instead of 4.
  - softmax normalization is DEFERRED: P = exp(S) is used unnormalized
    in P@V; the per-(head,query) denominators Z are computed head-major
    into a [128, 294] PSUM tile by 12 small ones-lhsT matmuls (rows
    32j = head 4q+j), one reciprocal, and the normalization is folded
    into the P@V PSUM eviction as a single [128, 294] multiply.  This
    removes all big [98, 1176] normalize ops from the critical DVE path.
  - v for all 24 pairs of a group is computed up front (group phase) so
    the per-pair loop only touches score/Z/PV/proj PSUM banks.
  - engine assignment spreads evictions: ACT does exp + av evictions,
    DVE does LN vector math + softmax recip/evict-mult, GPSIMD does LN
    scalar folds + qk/y evictions.
"""

import sys

if "/opt/trn_rl_repo" not in sys.path:
    sys.path.insert(0, "/opt/trn_rl_repo")

import numpy as np
import ml_dtypes

import concourse.bacc as bacc
import concourse.bass as bass
import concourse.tile as tile
import concourse.mybir as mybir
from concourse.bass_utils import run_bass_kernel_spmd

F32 = mybir.dt.float32
BF16 = mybir.dt.bfloat16

N_CORES = 8
B, T, H, W, D = 4, 6, 56, 56, 384
WSZ = 7
NH = 12
HD = D // NH            # 32
N = WSZ * WSZ           # 49 tokens / window
IMGS_CORE = (B * T) // N_CORES     # 3
TOK_CORE = IMGS_CORE * H * W       # 9408
NPAIR = TOK_CORE // (2 * N)        # 96 window pairs / core
PT = 2 * N                         # 98 tokens / pair
EPS = 1e-5

N_GROUPS = 4
PAIRS_G = NPAIR // N_GROUPS        # 24
TOK_G = PAIRS_G * PT               # 2352
CH = 4 * PT                        # 392-token chunks (stats + q/k)
NCH_G = TOK_G // CH                # 6 chunks / group
NCH = NPAIR * PT // CH             # 24 chunks / core
MASKVAL = -30000.0

Exp = mybir.ActivationFunctionType.Exp
Sqrt = mybir.ActivationFunctionType.Sqrt
Copy = mybir.ActivationFunctionType.Copy
MUL = mybir.AluOpType.mult
ADD = mybir.AluOpType.add
SUB = mybir.AluOpType.subtract


def _rel_index(w):
    coords = np.stack(np.meshgrid(np.arange(w), np.arange(w), indexing="ij")).reshape(2, -1)
    rel = coords[:, :, None] - coords[:, None, :]
    return (rel[0] + w - 1) * (2 * w - 1) + (rel[1] + w - 1)


def build_program(repeat=1, with_bias=False, n_stats=NCH, n_groups=N_GROUPS,
                  pairs_g=PAIRS_G, stage_upto=9, z_merge=False, qk_evict="act"):
    nc = bacc.Bacc("TRN2", target_bir_lowering=False, debug=False, num_devices=N_CORES)

    xT_d = nc.dram_tensor("xT", [D, TOK_CORE], BF16, kind="ExternalInput")
    qkw_d = nc.dram_tensor("qkw", [D, 2 * D], BF16, kind="ExternalInput")
    vw_d = nc.dram_tensor("vw", [D, D], BF16, kind="ExternalInput")
    pw_d = nc.dram_tensor("pw", [D, D], BF16, kind="ExternalInput")
    cqkr_d = nc.dram_tensor("cqkr", [2 * D], BF16, kind="ExternalInput")
    cvr_d = nc.dram_tensor("cvr", [D], BF16, kind="ExternalInput")
    pbr_d = nc.dram_tensor("pbr", [D], BF16, kind="ExternalInput")
    b2_d = nc.dram_tensor("b2", [4, PT, 3 * PT], BF16, kind="ExternalInput")
    i98_d = nc.dram_tensor("i98", [PT, PT], BF16, kind="ExternalInput")
    y_d = nc.dram_tensor("y", [TOK_CORE, D], F32, kind="ExternalOutput")

    from contextlib import ExitStack
    with tile.TileContext(nc) as tc, ExitStack() as ctx:
        const = ctx.enter_context(tc.tile_pool(name="const", bufs=1))
        grp = ctx.enter_context(tc.tile_pool(name="grp", bufs=2))
        avp = ctx.enter_context(tc.tile_pool(name="avp", bufs=2))
        wA = ctx.enter_context(tc.tile_pool(name="wA", bufs=3))
        wB = ctx.enter_context(tc.tile_pool(name="wB", bufs=2))
        wC = ctx.enter_context(tc.tile_pool(name="wC", bufs=2))
        ps_sc = ctx.enter_context(tc.tile_pool(name="ps_sc", bufs=4, space="PSUM"))
        ps_zo = ctx.enter_context(tc.tile_pool(name="ps_zo", bufs=2, space="PSUM"))
        ps_gen = ctx.enter_context(tc.tile_pool(name="ps_gen", bufs=2, space="PSUM"))

        # ---- resident constants -------------------------------------------------
        xT = [const.tile([128, TOK_CORE], BF16, name=f"xT{k}", tag=f"xT{k}") for k in range(3)]
        for k in range(3):
            nc.sync.dma_start(out=xT[k][:], in_=xT_d[128 * k:128 * (k + 1), :])
        qkw_sb = [const.tile([128, 2 * D], BF16, name=f"qkw{k}", tag=f"qkw{k}") for k in range(3)]
        vw_sb = [const.tile([128, D], BF16, name=f"vw{k}", tag=f"vw{k}") for k in range(3)]
        pw_sb = [const.tile([128, D], BF16, name=f"pw{k}", tag=f"pw{k}") for k in range(3)]
        for k in range(3):
            nc.sync.dma_start(out=qkw_sb[k][:], in_=qkw_d[128 * k:128 * (k + 1), :])
            nc.sync.dma_start(out=vw_sb[k][:], in_=vw_d[128 * k:128 * (k + 1), :])
            nc.sync.dma_start(out=pw_sb[k][:], in_=pw_d[128 * k:128 * (k + 1), :])
        cqkr_sb = const.tile([1, 2 * D], BF16, name="cqkr", tag="cqkr")
        nc.sync.dma_start(out=cqkr_sb[:], in_=cqkr_d[:])
        cvr_sb = const.tile([1, D], BF16, name="cvr", tag="cvr")
        nc.sync.dma_start(out=cvr_sb[:], in_=cvr_d[:])
        pbr_sb = const.tile([1, D], BF16, name="pbr", tag="pbr")
        nc.sync.dma_start(out=pbr_sb[:], in_=pbr_d[:])
        b2_sb = [const.tile([PT, 3 * PT], BF16, name=f"b2_{j}", tag=f"b2_{j}") for j in range(4)]
        for j in range(4):
            nc.sync.dma_start(out=b2_sb[j][:], in_=b2_d[j])
        i98_sb = const.tile([PT, PT], BF16, name="i98", tag="i98")
        nc.sync.dma_start(out=i98_sb[:], in_=i98_d[:])
        ones128 = const.tile([128, 128], BF16, name="ones128", tag="ones128")
        nc.vector.memset(ones128[:], 1.0)
        onesz = const.tile([PT, 32], BF16, name="onesz", tag="onesz")
        nc.vector.memset(onesz[:], 1.0)
        onesr_sb = const.tile([1, 512], BF16, name="onesr", tag="onesr")
        nc.vector.memset(onesr_sb[:], 1.0)

        rep_ctx = tc.For_i(0, repeat, 1) if repeat > 1 else None
        if rep_ctx is not None:
            rep_ctx.__enter__()

        # ---- stage 1: LN stats (broadcast form) + in-place normalize ------------
        for c in range(n_stats):
            c0 = CH * c
            zs = ps_sc.tile([128, CH], F32, name="zs", tag="sc")
            zq = ps_sc.tile([128, CH], F32, name="zq", tag="sc")
            sq = [wC.tile([128, CH], BF16, name=f"sq{k}", tag=f"sq{k}") for k in range(3)]
            for k in range(3):
                nc.vector.tensor_tensor(
                    out=sq[k][:], in0=xT[k][:, c0:c0 + CH], in1=xT[k][:, c0:c0 + CH],
                    op=MUL,
                )
                nc.tensor.matmul(zs[:], lhsT=ones128[:], rhs=xT[k][:, c0:c0 + CH],
                                 start=(k == 0), stop=(k == 2), tile_position=(0, 0))
            for k in range(3):
                nc.tensor.matmul(zq[:], lhsT=ones128[:], rhs=sq[k][:],
                                 start=(k == 0), stop=(k == 2), tile_position=(0, 0))
            # mu = zs/D (bf16, broadcast across partitions already)
            mu = wC.tile([128, CH], BF16, name="mu", tag="mu")
            nc.scalar.activation(out=mu[:], in_=zs[:], func=Copy, scale=1.0 / D)
            # v1 = zq/D + eps ; v2 = v1 - mu^2 ; isr = sqrt(1/v2)
            v1 = wC.tile([128, CH], F32, name="v1", tag="v1")
            nc.vector.tensor_scalar(out=v1[:], in0=zq[:], scalar1=1.0 / D, scalar2=EPS,
                                    op0=MUL, op1=ADD)
            m2 = wC.tile([128, CH], F32, name="m2", tag="m2")
            nc.vector.tensor_tensor(out=m2[:], in0=mu[:], in1=mu[:], op=MUL)
            nc.gpsimd.tensor_tensor(out=v1[:], in0=v1[:], in1=m2[:], op=SUB)
            nc.vector.reciprocal(out=v1[:], in_=v1[:])
            isr = wC.tile([128, CH], BF16, name="isr", tag="isr")
            nc.scalar.activation(out=isr[:], in_=v1[:], func=Sqrt)
            for k in range(3):
                nc.gpsimd.tensor_tensor(
                    out=xT[k][:, c0:c0 + CH], in0=xT[k][:, c0:c0 + CH],
                    in1=mu[:], op=SUB,
                )
                nc.vector.tensor_tensor(
                    out=xT[k][:, c0:c0 + CH], in0=xT[k][:, c0:c0 + CH],
                    in1=isr[:], op=MUL,
                )

        if stage_upto < 1:
            pass
        for g in range(n_groups if stage_upto >= 1 else 0):
            T0 = TOK_G * g
            qk = [grp.tile([128, TOK_G], BF16, name=f"qk{m}", tag=f"qk{m}") for m in range(6)]

            # ---- phase B: q/k projections (feature-major) ----------------------
            for cc in range(NCH_G):
                c0 = T0 + CH * cc
                for m in range(6):
                    pq = ps_gen.tile([128, CH], F32, name="pq", tag="gen")
                    if with_bias:
                        nc.tensor.matmul(
                            pq[:], lhsT=cqkr_sb[0:1, 128 * m:128 * (m + 1)],
                            rhs=onesr_sb[0:1, 0:CH],
                            start=True, stop=False, tile_position=(0, 0),
                        )
                    for k in range(3):
                        nc.tensor.matmul(
                            pq[:],
                            lhsT=qkw_sb[k][:, 128 * m:128 * (m + 1)],
                            rhs=xT[k][:, c0:c0 + CH],
                            start=(k == 0 and not with_bias), stop=(k == 2),
                            tile_position=(0, 0),
                        )
                    if qk_evict == "act" or m < 3:
                        nc.scalar.activation(out=qk[m][:, CH * cc:CH * (cc + 1)],
                                             in_=pq[:], func=Copy)
                    else:
                        nc.vector.tensor_copy(out=qk[m][:, CH * cc:CH * (cc + 1)],
                                              in_=pq[:])

            # ---- phase B2: v for all pairs of the group (token-major) ----------
            av_g = avp.tile([PT, pairs_g * D], BF16, name="av", tag="av")
            for p in range(pairs_g):
                t0 = T0 + PT * p
                pv = ps_gen.tile([PT, D], F32, name="pv", tag="gen")
                if with_bias:
                    nc.tensor.matmul(pv[:], lhsT=onesr_sb[0:1, 0:PT], rhs=cvr_sb[:],
                                     start=True, stop=False, tile_position=(0, 0))
                for k in range(3):
                    nc.tensor.matmul(
                        pv[:], lhsT=xT[k][:, t0:t0 + PT], rhs=vw_sb[k][:],
                        start=(k == 0 and not with_bias), stop=(k == 2),
                        tile_position=(0, 0),
                    )
                nc.scalar.activation(out=av_g[:, D * p:D * (p + 1)], in_=pv[:], func=Copy)

            if stage_upto < 2:
                continue
            # ---- phase C: per window pair --------------------------------------
            for p in range(pairs_g):
                qc = PT * p           # within-group token col (qk)

                # scores, strip-major: PSUM bank j holds heads {j, 4+j, 8+j};
                # each bank is only written by PE row-band 32j (HW restriction:
                # two row tiles must not write one PSUM bank concurrently)
                sc_j = []
                for j in range(4):
                    sc = ps_sc.tile([PT, 3 * PT], F32, name="sc", tag="sc")
                    nc.tensor.matmul(sc[:], lhsT=i98_sb[:], rhs=b2_sb[j][:],
                                     start=True, stop=False, tile_position=(0, 0))
                    sc_j.append(sc)
                for j in range(4):
                    hb = 32 * j
                    for quad in range(3):
                        nc.tensor.matmul(
                            sc_j[j][:, PT * quad:PT * (quad + 1)],
                            lhsT=qk[3 + quad][hb:hb + 32, qc:qc + PT],
                            rhs=qk[quad][hb:hb + 32, qc:qc + PT],
                            start=False, stop=(quad == 2),
                            tile_position=(hb, 0),
                        )
                if stage_upto < 3:
                    continue
                p_t = wA.tile([PT, NH * PT], BF16, name="pt", tag="pt")
                for j in range(4):
                    nc.scalar.activation(
                        out=p_t[:, 3 * PT * j:3 * PT * (j + 1)], in_=sc_j[j][:],
                        func=Exp,
                    )

                if stage_upto < 4:
                    continue
                # Z head-major: rows 32*jj = heads {jj, 4+jj, 8+jj}; the
                # strip-major p_t layout makes each row-band's 3 quads one
                # contiguous 294-col rhs -> one matmul per band.
                zqp = ps_zo.tile([128, 512], F32, name="zq2", tag="zo")
                if z_merge:
                    for jj in range(4):
                        nc.tensor.matmul(
                            zqp[32 * jj:32 * (jj + 1), 0:3 * PT],
                            lhsT=onesz[:],
                            rhs=p_t[:, 3 * PT * jj:3 * PT * (jj + 1)],
                            start=True, stop=True, tile_position=(0, 32 * jj),
                        )
                else:
                    for quad in range(3):
                        for jj in range(4):
                            h = 4 * quad + jj
                            pcol = 3 * PT * (h % 4) + PT * (h // 4)
                            nc.tensor.matmul(
                                zqp[32 * jj:32 * (jj + 1), PT * quad:PT * (quad + 1)],
                                lhsT=onesz[:],
                                rhs=p_t[:, pcol:pcol + PT],
                                start=True, stop=True, tile_position=(0, 32 * jj),
                            )
                rz = wB.tile([128, 3 * PT], BF16, name="rz", tag="rz")
                with nc.allow_low_precision(reason="1/Z bf16; feeds bf16 multiply"):
                    nc.vector.reciprocal(out=rz[:], in_=zqp[:, 0:3 * PT])

                if stage_upto < 5:
                    continue
                # P@V transposed, unnormalized; same head-major packing as Z
                ao = ps_zo.tile([128, 512], F32, name="ao", tag="zo")
                for quad in range(3):
                    for jj in range(4):
                        h = 4 * quad + jj
                        pcol = 3 * PT * (h % 4) + PT * (h // 4)
                        nc.tensor.matmul(
                            ao[32 * jj:32 * (jj + 1), PT * quad:PT * (quad + 1)],
                            lhsT=av_g[:, D * p + HD * h:D * p + HD * (h + 1)],
                            rhs=p_t[:, pcol:pcol + PT],
                            start=True, stop=True, tile_position=(0, 32 * jj),
                        )
                # normalization folded into the eviction
                atT = wB.tile([128, 3 * PT], BF16, name="atT", tag="atT")
                nc.vector.tensor_tensor(out=atT[:], in0=ao[:, 0:3 * PT], in1=rz[:], op=MUL)

                if stage_upto < 6:
                    continue
                # proj (+pb via rank-1) + store
                pp = ps_gen.tile([PT, D], F32, name="pp", tag="gen")
                if with_bias:
                    nc.tensor.matmul(pp[:], lhsT=onesr_sb[0:1, 0:PT], rhs=pbr_sb[:],
                                     start=True, stop=False, tile_position=(0, 0))
                for quad in range(3):
                    nc.tensor.matmul(
                        pp[:], lhsT=atT[:, PT * quad:PT * (quad + 1)], rhs=pw_sb[quad][:],
                        start=(quad == 0 and not with_bias), stop=(quad == 2),
                        tile_position=(0, 0),
                    )
                y_sb = wC.tile([PT, D], F32, name="y", tag="y")
                nc.vector.tensor_copy(out=y_sb[:], in_=pp[:])
                r0 = T0 + PT * p
                nc.sync.dma_start(out=y_d[r0:r0 + PT, :], in_=y_sb[:])

        if rep_ctx is not None:
            rep_ctx.__exit__(None, None, None)

    nc.compile()
    return nc


_NC_CACHE = {}


def _get_program(with_bias=False):
    key = ("nc", with_bias)
    if key not in _NC_CACHE:
        _NC_CACHE[key] = build_program(with_bias=with_bias)
    return _NC_CACHE[key]


def _window_order(xf):
    BT = xf.shape[0]
    x6 = xf.reshape(BT, H // WSZ, WSZ, W // WSZ, WSZ, D)
    return np.ascontiguousarray(x6.transpose(0, 1, 3, 2, 4, 5)).reshape(-1, D)


def _window_unorder(yw):
    BT = B * T
    y6 = yw.reshape(BT, H // WSZ, W // WSZ, WSZ, WSZ, D)
    return np.ascontiguousarray(y6.transpose(0, 1, 3, 2, 4, 5)).reshape(BT, H, W, D)


def prepare_inputs(x, ln_g, ln_b, qkv_w, qkv_b, proj_w, proj_b, rel_bias_table):
    x = np.asarray(x, np.float32)
    ln_g = np.asarray(ln_g, np.float32)
    ln_b = np.asarray(ln_b, np.float32)
    qkv_w = np.asarray(qkv_w, np.float32)
    qkv_b = np.asarray(qkv_b, np.float32)
    proj_w = np.asarray(proj_w, np.float32)
    proj_b = np.asarray(proj_b, np.float32)
    rel_bias_table = np.asarray(rel_bias_table, np.float32)

    scale = HD ** -0.5
    wq = qkv_w[:, :D] * ln_g[:, None] * scale
    wk = qkv_w[:, D:2 * D] * ln_g[:, None]
    wv = qkv_w[:, 2 * D:] * ln_g[:, None]
    cq = (ln_b @ qkv_w[:, :D] + qkv_b[:D]) * scale
    ck = ln_b @ qkv_w[:, D:2 * D] + qkv_b[D:2 * D]
    cv = ln_b @ qkv_w[:, 2 * D:] + qkv_b[2 * D:]

    qkw = np.concatenate([wq, wk], axis=1).astype(ml_dtypes.bfloat16)
    cqk = np.concatenate([cq, ck]).astype(np.float32)

    idx = _rel_index(WSZ)
    bias = rel_bias_table[idx.reshape(-1)].reshape(N, N, NH)  # [q, k, h]
    # strip-major bias tables: table j holds heads {j, 4+j, 8+j}
    b2 = np.full((4, PT, 3 * PT), MASKVAL, np.float32)
    for j in range(4):
        for quad in range(3):
            h = 4 * quad + j
            blkT = bias[:, :, h].T  # [k, q]
            for w in range(2):
                b2[j, N * w:N * (w + 1), PT * quad + N * w:PT * quad + N * (w + 1)] = blkT

    xw = _window_order(x.reshape(B * T, H, W, D))

    common = {
        "qkw": qkw,
        "vw": wv.astype(ml_dtypes.bfloat16),
        "pw": proj_w.astype(ml_dtypes.bfloat16),
        "cqkr": cqk.astype(ml_dtypes.bfloat16),
        "cvr": cv.astype(ml_dtypes.bfloat16),
        "pbr": proj_b.astype(ml_dtypes.bfloat16),
        "b2": b2.astype(ml_dtypes.bfloat16),
        "i98": np.eye(PT, dtype=np.float32).astype(ml_dtypes.bfloat16),
    }
    in_maps = []
    for c in range(N_CORES):
        m = dict(common)
        xc = xw[TOK_CORE * c:TOK_CORE * (c + 1)]
        m["xT"] = np.ascontiguousarray(xc.T).astype(ml_dtypes.bfloat16)
        in_maps.append(m)
    return in_maps


def kernel(x, ln_g, ln_b, qkv_w, qkv_b, proj_w, proj_b, rel_bias_table):
    in_maps = prepare_inputs(x, ln_g, ln_b, qkv_w, qkv_b, proj_w, proj_b, rel_bias_table)
    with_bias = any(
        np.any(np.asarray(in_maps[0][k], np.float32) != 0.0)
        for k in ("cqkr", "cvr", "pbr")
    )
    nc = _get_program(with_bias=with_bias)
    res = run_bass_kernel_spmd(nc, in_maps, core_ids=list(range(N_CORES)))
    yw = np.concatenate([res.results[c]["y"] for c in range(N_CORES)], axis=0)
    out = _window_unorder(yw).reshape(B, T, H, W, D)
    return out.astype(np.float32)


# revision 3
# speedup vs baseline: 10612.2049x; 1.4142x over previous
"""Swin-style windowed local self-attention (LN -> QKV -> 7x7 window MHA
with relative position bias -> proj) on 8 Trainium2 NeuronCores.

Sharding: pure data parallel over B*T (24 images -> 3 per core).

v3 design -- engine-balance rework of v2.  Differences:
  - LN stats are computed in BROADCAST form: all-ones [128,128] lhsT
    matmuls give per-token sum / sumsq replicated across all 128
    partitions directly in PSUM, so the per-token mean/var math runs as
    [128, 392] multi-lane ops (no single-lane [1,N] row ops, no gpsimd
    partition broadcasts).
  - scores use 3 PSUM banks of [98, 392] (bank j holds heads {j, 3+j,
    6+j, 9+j}); bias+mask is accumulated first via an identity-weight
    matmul, 3 exp ops per pair instead of 4.
  - softmax normalization is DEFERRED: P = exp(S) is used unnormalized
    in P@V; the per-(head,query) denominators Z are computed head-major
    into a [128, 294] PSUM tile by 12 small ones-lhsT matmuls (rows
    32j = head 4q+j), one reciprocal, and the normalization is folded
    into the P@V PSUM eviction as a single [128, 294] multiply.  This
    removes all big [98, 1176] normalize ops from the critical DVE path.
  - v for all 24 pairs of a group is computed up front (group phase) so
    the per-pair loop only touches score/Z/PV/proj PSUM banks.
  - engine assignment spreads evictions: ACT does exp + qk/av
    evictions, DVE does LN vector math + softmax recip/evict-mult +
    y evictions, GPSIMD does LN scalar folds.
  - phase C is SOFTWARE-PIPELINED: pair p+1's score matmuls are emitted
    before pair p's Z/PV/proj so the in-order PE queue never stalls on
    the exp (ACT); the relative-position bias is applied as a
    precomputed exp(bias) multiply on DVE (exp(s+b) = exp(s)*exp(b),
    exact cross-window masking since exp(-30000) == 0), which removes
    the per-pair bias matmuls from the PE.
"""

import sys

if "/opt/trn_rl_repo" not in sys.path:
    sys.path.insert(0, "/opt/trn_rl_repo")

import numpy as np
import ml_dtypes

import concourse.bacc as bacc
import concourse.bass as bass
import concourse.tile as tile
import concourse.mybir as mybir
from concourse.bass_utils import run_bass_kernel_spmd

F32 = mybir.dt.float32
BF16 = mybir.dt.bfloat16

N_CORES = 8
B, T, H, W, D = 4, 6, 56, 56, 384
WSZ = 7
NH = 12
HD = D // NH            # 32
N = WSZ * WSZ           # 49 tokens / window
IMGS_CORE = (B * T) // N_CORES     # 3
TOK_CORE = IMGS_CORE * H * W       # 9408
NPAIR = TOK_CORE // (2 * N)        # 96 window pairs / core
PT = 2 * N                         # 98 tokens / pair
EPS = 1e-5

N_GROUPS = 4
PAIRS_G = NPAIR // N_GROUPS        # 24
TOK_G = PAIRS_G * PT               # 2352
CH = 4 * PT                        # 392-token chunks (stats + q/k)
NCH_G = TOK_G // CH                # 6 chunks / group
NCH = NPAIR * PT // CH             # 24 chunks / core
MASKVAL = -30000.0

Exp = mybir.ActivationFunctionType.Exp
Sqrt = mybir.ActivationFunctionType.Sqrt
Copy = mybir.ActivationFunctionType.Copy
MUL = mybir.AluOpType.mult
ADD = mybir.AluOpType.add
SUB = mybir.AluOpType.subtract


def _rel_index(w):
    coords = np.stack(np.meshgrid(np.arange(w), np.arange(w), indexing="ij")).reshape(2, -1)
    rel = coords[:, :, None] - coords[:, None, :]
    return (rel[0] + w - 1) * (2 * w - 1) + (rel[1] + w - 1)


def build_program(repeat=1, with_bias=False, n_stats=NCH, n_groups=N_GROUPS,
                  pairs_g=PAIRS_G, stage_upto=9, z_merge=False, qk_evict="act",
                  bias_mode="expb_dve"):
    nc = bacc.Bacc("TRN2", target_bir_lowering=False, debug=False, num_devices=N_CORES)

    xT_d = nc.dram_tensor("xT", [D, TOK_CORE], BF16, kind="ExternalInput")
    qkw_d = nc.dram_tensor("qkw", [D, 2 * D], BF16, kind="ExternalInput")
    vw_d = nc.dram_tensor("vw", [D, D], BF16, kind="ExternalInput")
    pw_d = nc.dram_tensor("pw", [D, D], BF16, kind="ExternalInput")
    cqkr_d = nc.dram_tensor("cqkr", [2 * D], BF16, kind="ExternalInput")
    cvr_d = nc.dram_tensor("cvr", [D], BF16, kind="ExternalInput")
    pbr_d = nc.dram_tensor("pbr", [D], BF16, kind="ExternalInput")
    b2_d = nc.dram_tensor("b2", [4, PT, 3 * PT], BF16, kind="ExternalInput")
    expb_d = nc.dram_tensor("expb", [PT, NH * PT], BF16, kind="ExternalInput")
    i98_d = nc.dram_tensor("i98", [PT, PT], BF16, kind="ExternalInput")
    y_d = nc.dram_tensor("y", [TOK_CORE, D], F32, kind="ExternalOutput")

    from contextlib import ExitStack
    with tile.TileContext(nc) as tc, ExitStack() as ctx:
        const = ctx.enter_context(tc.tile_pool(name="const", bufs=1))
        grp = ctx.enter_context(tc.tile_pool(name="grp", bufs=2))
        avp = ctx.enter_context(tc.tile_pool(name="avp", bufs=2))
        wA = ctx.enter_context(tc.tile_pool(name="wA", bufs=3))
        wB = ctx.enter_context(tc.tile_pool(name="wB", bufs=2))
        wC = ctx.enter_context(tc.tile_pool(name="wC", bufs=2))
        ps_sc = ctx.enter_context(tc.tile_pool(name="ps_sc", bufs=4, space="PSUM"))
        ps_zo = ctx.enter_context(tc.tile_pool(name="ps_zo", bufs=2, space="PSUM"))
        ps_gen = ctx.enter_context(tc.tile_pool(name="ps_gen", bufs=2, space="PSUM"))

        # ---- resident constants -------------------------------------------------
        xT = [const.tile([128, TOK_CORE], BF16, name=f"xT{k}", tag=f"xT{k}") for k in range(3)]
        for k in range(3):
            nc.sync.dma_start(out=xT[k][:], in_=xT_d[128 * k:128 * (k + 1), :])
        qkw_sb = [const.tile([128, 2 * D], BF16, name=f"qkw{k}", tag=f"qkw{k}") for k in range(3)]
        vw_sb = [const.tile([128, D], BF16, name=f"vw{k}", tag=f"vw{k}") for k in range(3)]
        pw_sb = [const.tile([128, D], BF16, name=f"pw{k}", tag=f"pw{k}") for k in range(3)]
        for k in range(3):
            nc.sync.dma_start(out=qkw_sb[k][:], in_=qkw_d[128 * k:128 * (k + 1), :])
            nc.sync.dma_start(out=vw_sb[k][:], in_=vw_d[128 * k:128 * (k + 1), :])
            nc.sync.dma_start(out=pw_sb[k][:], in_=pw_d[128 * k:128 * (k + 1), :])
        cqkr_sb = const.tile([1, 2 * D], BF16, name="cqkr", tag="cqkr")
        nc.sync.dma_start(out=cqkr_sb[:], in_=cqkr_d[:])
        cvr_sb = const.tile([1, D], BF16, name="cvr", tag="cvr")
        nc.sync.dma_start(out=cvr_sb[:], in_=cvr_d[:])
        pbr_sb = const.tile([1, D], BF16, name="pbr", tag="pbr")
        nc.sync.dma_start(out=pbr_sb[:], in_=pbr_d[:])
        b2_sb = [const.tile([PT, 3 * PT], BF16, name=f"b2_{j}", tag=f"b2_{j}") for j in range(4)]
        for j in range(4):
            nc.sync.dma_start(out=b2_sb[j][:], in_=b2_d[j])
        i98_sb = const.tile([PT, PT], BF16, name="i98", tag="i98")
        nc.sync.dma_start(out=i98_sb[:], in_=i98_d[:])
        expb_sb = const.tile([PT, NH * PT], BF16, name="expb", tag="expb")
        nc.sync.dma_start(out=expb_sb[:], in_=expb_d[:])
        ones128 = const.tile([128, 128], BF16, name="ones128", tag="ones128")
        nc.vector.memset(ones128[:], 1.0)
        onesz = const.tile([PT, 32], BF16, name="onesz", tag="onesz")
        nc.vector.memset(onesz[:], 1.0)
        onesr_sb = const.tile([1, 512], BF16, name="onesr", tag="onesr")
        nc.vector.memset(onesr_sb[:], 1.0)

        rep_ctx = tc.For_i(0, repeat, 1) if repeat > 1 else None
        if rep_ctx is not None:
            rep_ctx.__enter__()

        # ---- stage 1: LN stats (broadcast form) + in-place normalize ------------
        for c in range(n_stats):
            c0 = CH * c
            zs = ps_sc.tile([128, CH], F32, name="zs", tag="sc")
            zq = ps_sc.tile([128, CH], F32, name="zq", tag="sc")
            sq = [wC.tile([128, CH], BF16, name=f"sq{k}", tag=f"sq{k}") for k in range(3)]
            for k in range(3):
                nc.vector.tensor_tensor(
                    out=sq[k][:], in0=xT[k][:, c0:c0 + CH], in1=xT[k][:, c0:c0 + CH],
                    op=MUL,
                )
                nc.tensor.matmul(zs[:], lhsT=ones128[:], rhs=xT[k][:, c0:c0 + CH],
                                 start=(k == 0), stop=(k == 2), tile_position=(0, 0))
            for k in range(3):
                nc.tensor.matmul(zq[:], lhsT=ones128[:], rhs=sq[k][:],
                                 start=(k == 0), stop=(k == 2), tile_position=(0, 0))
            # mu = zs/D (bf16, broadcast across partitions already)
            mu = wC.tile([128, CH], BF16, name="mu", tag="mu")
            nc.scalar.activation(out=mu[:], in_=zs[:], func=Copy, scale=1.0 / D)
            # v1 = zq/D + eps ; v2 = v1 - mu^2 ; isr = sqrt(1/v2)
            v1 = wC.tile([128, CH], F32, name="v1", tag="v1")
            nc.vector.tensor_scalar(out=v1[:], in0=zq[:], scalar1=1.0 / D, scalar2=EPS,
                                    op0=MUL, op1=ADD)
            m2 = wC.tile([128, CH], F32, name="m2", tag="m2")
            nc.vector.tensor_tensor(out=m2[:], in0=mu[:], in1=mu[:], op=MUL)
            nc.gpsimd.tensor_tensor(out=v1[:], in0=v1[:], in1=m2[:], op=SUB)
            nc.vector.reciprocal(out=v1[:], in_=v1[:])
            isr = wC.tile([128, CH], BF16, name="isr", tag="isr")
            nc.scalar.activation(out=isr[:], in_=v1[:], func=Sqrt)
            for k in range(3):
                nc.gpsimd.tensor_tensor(
                    out=xT[k][:, c0:c0 + CH], in0=xT[k][:, c0:c0 + CH],
                    in1=mu[:], op=SUB,
                )
                nc.vector.tensor_tensor(
                    out=xT[k][:, c0:c0 + CH], in0=xT[k][:, c0:c0 + CH],
                    in1=isr[:], op=MUL,
                )

        if stage_upto < 1:
            pass
        for g in range(n_groups if stage_upto >= 1 else 0):
            T0 = TOK_G * g
            qk = [grp.tile([128, TOK_G], BF16, name=f"qk{m}", tag=f"qk{m}") for m in range(6)]

            # ---- phase B: q/k projections (feature-major) ----------------------
            for cc in range(NCH_G):
                c0 = T0 + CH * cc
                for m in range(6):
                    pq = ps_gen.tile([128, CH], F32, name="pq", tag="gen")
                    if with_bias:
                        nc.tensor.matmul(
                            pq[:], lhsT=cqkr_sb[0:1, 128 * m:128 * (m + 1)],
                            rhs=onesr_sb[0:1, 0:CH],
                            start=True, stop=False, tile_position=(0, 0),
                        )
                    for k in range(3):
                        nc.tensor.matmul(
                            pq[:],
                            lhsT=qkw_sb[k][:, 128 * m:128 * (m + 1)],
                            rhs=xT[k][:, c0:c0 + CH],
                            start=(k == 0 and not with_bias), stop=(k == 2),
                            tile_position=(0, 0),
                        )
                    if qk_evict == "act" or m < 3:
                        nc.scalar.activation(out=qk[m][:, CH * cc:CH * (cc + 1)],
                                             in_=pq[:], func=Copy)
                    else:
                        nc.vector.tensor_copy(out=qk[m][:, CH * cc:CH * (cc + 1)],
                                              in_=pq[:])

            # ---- phase B2: v for all pairs of the group (token-major) ----------
            av_g = avp.tile([PT, pairs_g * D], BF16, name="av", tag="av")
            for p in range(pairs_g):
                t0 = T0 + PT * p
                pv = ps_gen.tile([PT, D], F32, name="pv", tag="gen")
                if with_bias:
                    nc.tensor.matmul(pv[:], lhsT=onesr_sb[0:1, 0:PT], rhs=cvr_sb[:],
                                     start=True, stop=False, tile_position=(0, 0))
                for k in range(3):
                    nc.tensor.matmul(
                        pv[:], lhsT=xT[k][:, t0:t0 + PT], rhs=vw_sb[k][:],
                        start=(k == 0 and not with_bias), stop=(k == 2),
                        tile_position=(0, 0),
                    )
                nc.scalar.activation(out=av_g[:, D * p:D * (p + 1)], in_=pv[:], func=Copy)

            if stage_upto < 2:
                continue
            # ---- phase C: per window pair, SOFTWARE-PIPELINED ------------------
            # The PE executes its queue in order; Z(p) depends on exp(p) (ACT),
            # so emitting Z right after the score matmuls stalls the PE for the
            # whole exp.  Instead emit pair p+1's bias/score matmuls BEFORE
            # pair p's Z/PV/proj so the PE always has exp-independent work.
            use_mm = bias_mode == "mm"
            state = {}

            def emit_front(p):
                qc = PT * p
                sc_j = []
                for j in range(4):
                    sc = ps_sc.tile([PT, 3 * PT], F32, name="sc", tag="sc")
                    if use_mm:
                        nc.tensor.matmul(sc[:], lhsT=i98_sb[:], rhs=b2_sb[j][:],
                                         start=True, stop=False, tile_position=(0, 0))
                    sc_j.append(sc)
                for j in range(4):
                    hb = 32 * j
                    for quad in range(3):
                        nc.tensor.matmul(
                            sc_j[j][:, PT * quad:PT * (quad + 1)],
                            lhsT=qk[3 + quad][hb:hb + 32, qc:qc + PT],
                            rhs=qk[quad][hb:hb + 32, qc:qc + PT],
                            start=(quad == 0 and not use_mm), stop=(quad == 2),
                            tile_position=(hb, 0),
                        )
                if stage_upto < 3:
                    return
                p_t = wA.tile([PT, NH * PT], BF16, name="pt", tag="pt")
                for j in range(4):
                    nc.scalar.activation(
                        out=p_t[:, 3 * PT * j:3 * PT * (j + 1)], in_=sc_j[j][:],
                        func=Exp,
                    )
                if bias_mode == "expb_pool":
                    for hf in range(2):
                        cl = 6 * PT * hf
                        nc.gpsimd.tensor_tensor(
                            out=p_t[:, cl:cl + 6 * PT], in0=p_t[:, cl:cl + 6 * PT],
                            in1=expb_sb[:, cl:cl + 6 * PT], op=MUL,
                        )
                elif bias_mode == "expb_dve":
                    for hf in range(2):
                        cl = 6 * PT * hf
                        nc.vector.tensor_tensor(
                            out=p_t[:, cl:cl + 6 * PT], in0=p_t[:, cl:cl + 6 * PT],
                            in1=expb_sb[:, cl:cl + 6 * PT], op=MUL,
                        )
                elif bias_mode == "expb_split":
                    nc.vector.tensor_tensor(
                        out=p_t[:, 0:6 * PT], in0=p_t[:, 0:6 * PT],
                        in1=expb_sb[:, 0:6 * PT], op=MUL,
                    )
                    nc.gpsimd.tensor_tensor(
                        out=p_t[:, 6 * PT:12 * PT], in0=p_t[:, 6 * PT:12 * PT],
                        in1=expb_sb[:, 6 * PT:12 * PT], op=MUL,
                    )
                state[p] = p_t

            def emit_back(p):
                if stage_upto < 4 or p not in state:
                    return
                p_t = state.pop(p)
                zqp = ps_zo.tile([128, 512], F32, name="zq2", tag="zo")
                if z_merge:
                    for jj in range(4):
                        nc.tensor.matmul(
                            zqp[32 * jj:32 * (jj + 1), 0:3 * PT],
                            lhsT=onesz[:],
                            rhs=p_t[:, 3 * PT * jj:3 * PT * (jj + 1)],
                            start=True, stop=True, tile_position=(0, 32 * jj),
                        )
                else:
                    for quad in range(3):
                        for jj in range(4):
                            h = 4 * quad + jj
                            pcol = 3 * PT * (h % 4) + PT * (h // 4)
                            nc.tensor.matmul(
                                zqp[32 * jj:32 * (jj + 1), PT * quad:PT * (quad + 1)],
                                lhsT=onesz[:],
                                rhs=p_t[:, pcol:pcol + PT],
                                start=True, stop=True, tile_position=(0, 32 * jj),
                            )
                rz = wB.tile([128, 3 * PT], BF16, name="rz", tag="rz")
                with nc.allow_low_precision(reason="1/Z bf16; feeds bf16 multiply"):
                    nc.vector.reciprocal(out=rz[:], in_=zqp[:, 0:3 * PT])

                if stage_upto < 5:
                    return
                ao = ps_zo.tile([128, 512], F32, name="ao", tag="zo")
                for quad in range(3):
                    for jj in range(4):
                        h = 4 * quad + jj
                        pcol = 3 * PT * (h % 4) + PT * (h // 4)
                        nc.tensor.matmul(
                            ao[32 * jj:32 * (jj + 1), PT * quad:PT * (quad + 1)],
                            lhsT=av_g[:, D * p + HD * h:D * p + HD * (h + 1)],
                            rhs=p_t[:, pcol:pcol + PT],
                            start=True, stop=True, tile_position=(0, 32 * jj),
                        )
                atT = wB.tile([128, 3 * PT], BF16, name="atT", tag="atT")
                nc.vector.tensor_tensor(out=atT[:], in0=ao[:, 0:3 * PT], in1=rz[:], op=MUL)

                if stage_upto < 6:
                    return
                pp = ps_gen.tile([PT, D], F32, name="pp", tag="gen")
                if with_bias:
                    nc.tensor.matmul(pp[:], lhsT=onesr_sb[0:1, 0:PT], rhs=pbr_sb[:],
                                     start=True, stop=False, tile_position=(0, 0))
                for quad in range(3):
                    nc.tensor.matmul(
                        pp[:], lhsT=atT[:, PT * quad:PT * (quad + 1)], rhs=pw_sb[quad][:],
                        start=(quad == 0 and not with_bias), stop=(quad == 2),
                        tile_position=(0, 0),
                    )
                y_sb = wC.tile([PT, D], F32, name="y", tag="y")
                nc.vector.tensor_copy(out=y_sb[:], in_=pp[:])
                r0 = T0 + PT * p
                nc.sync.dma_start(out=y_d[r0:r0 + PT, :], in_=y_sb[:])

            for p in range(pairs_g):
                emit_front(p)
                if p > 0:
                    emit_back(p - 1)
            emit_back(pairs_g - 1)

        if rep_ctx is not None:
            rep_ctx.__exit__(None, None, None)

    nc.compile()
    return nc


_NC_CACHE = {}


def _get_program(with_bias=False):
    key = ("nc", with_bias)
    if key not in _NC_CACHE:
        _NC_CACHE[key] = build_program(with_bias=with_bias)
    return _NC_CACHE[key]


def _window_order(xf):
    BT = xf.shape[0]
    x6 = xf.reshape(BT, H // WSZ, WSZ, W // WSZ, WSZ, D)
    return np.ascontiguousarray(x6.transpose(0, 1, 3, 2, 4, 5)).reshape(-1, D)


def _window_unorder(yw):
    BT = B * T
    y6 = yw.reshape(BT, H // WSZ, W // WSZ, WSZ, WSZ, D)
    return np.ascontiguousarray(y6.transpose(0, 1, 3, 2, 4, 5)).reshape(BT, H, W, D)


def prepare_inputs(x, ln_g, ln_b, qkv_w, qkv_b, proj_w, proj_b, rel_bias_table):
    x = np.asarray(x, np.float32)
    ln_g = np.asarray(ln_g, np.float32)
    ln_b = np.asarray(ln_b, np.float32)
    qkv_w = np.asarray(qkv_w, np.float32)
    qkv_b = np.asarray(qkv_b, np.float32)
    proj_w = np.asarray(proj_w, np.float32)
    proj_b = np.asarray(proj_b, np.float32)
    rel_bias_table = np.asarray(rel_bias_table, np.float32)

    scale = HD ** -0.5
    wq = qkv_w[:, :D] * ln_g[:, None] * scale
    wk = qkv_w[:, D:2 * D] * ln_g[:, None]
    wv = qkv_w[:, 2 * D:] * ln_g[:, None]
    cq = (ln_b @ qkv_w[:, :D] + qkv_b[:D]) * scale
    ck = ln_b @ qkv_w[:, D:2 * D] + qkv_b[D:2 * D]
    cv = ln_b @ qkv_w[:, 2 * D:] + qkv_b[2 * D:]

    qkw = np.concatenate([wq, wk], axis=1).astype(ml_dtypes.bfloat16)
    cqk = np.concatenate([cq, ck]).astype(np.float32)

    idx = _rel_index(WSZ)
    bias = rel_bias_table[idx.reshape(-1)].reshape(N, N, NH)  # [q, k, h]
    # strip-major bias tables: table j holds heads {j, 4+j, 8+j}
    b2 = np.full((4, PT, 3 * PT), MASKVAL, np.float32)
    for j in range(4):
        for quad in range(3):
            h = 4 * quad + j
            blkT = bias[:, :, h].T  # [k, q]
            for w in range(2):
                b2[j, N * w:N * (w + 1), PT * quad + N * w:PT * quad + N * (w + 1)] = blkT

    xw = _window_order(x.reshape(B * T, H, W, D))

    common = {
        "qkw": qkw,
        "vw": wv.astype(ml_dtypes.bfloat16),
        "pw": proj_w.astype(ml_dtypes.bfloat16),
        "cqkr": cqk.astype(ml_dtypes.bfloat16),
        "cvr": cv.astype(ml_dtypes.bfloat16),
        "pbr": proj_b.astype(ml_dtypes.bfloat16),
        "b2": b2.astype(ml_dtypes.bfloat16),
        "expb": np.exp(np.concatenate([b2[j] for j in range(4)], axis=1)).astype(ml_dtypes.bfloat16),
        "i98": np.eye(PT, dtype=np.float32).astype(ml_dtypes.bfloat16),
    }
    in_maps = []
    for c in range(N_CORES):
        m = dict(common)
        xc = xw[TOK_CORE * c:TOK_CORE * (c + 1)]
        m["xT"] = np.ascontiguousarray(xc.T).astype(ml_dtypes.bfloat16)
        in_maps.append(m)
    return in_maps


def kernel(x, ln_g, ln_b, qkv_w, qkv_b, proj_w, proj_b, rel_bias_table):
    in_maps = prepare_inputs(x, ln_g, ln_b, qkv_w, qkv_b, proj_w, proj_b, rel_bias_table)
    with_bias = any(
        np.any(np.asarray(in_maps[0][k], np.float32) != 0.0)
        for k in ("cqkr", "cvr", "pbr")
    )
    nc = _get_program(with_bias=with_bias)
    res = run_bass_kernel_spmd(nc, in_maps, core_ids=list(range(N_CORES)))
    yw = np.concatenate([res.results[c]["y"] for c in range(N_CORES)], axis=0)
    out = _window_unorder(yw).reshape(B, T, H, W, D)
    return out.astype(np.float32)
